# revision 26
# baseline (speedup 1.0000x reference)
"""Banded-matrix matmul kernel for Trainium2, SPMD over 8 NeuronCores.

Problem: out[b,s,o] = sum_i x[b,s,i] * W[o,i] + bias[o] with W a 4096x4096
band matrix (bandwidth 512 -> W[o,i] != 0 iff |o-i| <= 512), given in COO
form (W_values, rows, cols) with deterministic band ordering.

Strategy:
  - Host: densify W; shard tokens 8-way (data parallel; band + bias
    replicated). All device-side tensors are host-packed partition-major so
    every DMA is a 2D pattern with 6-8KB contiguous per-partition rows.
  - Device (per core): out.T[o,s] = W @ x.T per 128-row output tile,
    accumulating over the band's k-tiles (block tridiagonal in 512-blocks)
    in fp32 PSUM. Interior o-tiles (t=4..27) have exactly two TRIANGULAR
    k-tiles (t-4 upper-tri, t+4 lower-tri, ~129/1025 of each row's terms);
    those two are computed as ONE fp8e4 DoubleRow matmul pair (2 contraction
    planes packed along the free axis) while the 7 dense k-tiles stay fp16.
    Measured on the real data this puts rel err at 1.5e-2 (gate 2e-2) and
    removes 2 of 18 512-cycle PE passes per interior o-tile (~9% of the PE
    stream, more if DoubleRow really is 0.5 cycles/row as the cost model
    says). Output drains to fp16 (adds <5e-4 err), halving store traffic
    and the final-store landing that gates the kernel tail.
  - Host: unpack per-core [128, 32*1024] fp16 outputs back to [B, S, 4096]
    fp32.

Measured on 8 axon-tunneled trn2 cores: ~125-128us (vs 141us fp16
baseline), rel err 1.495e-2. Anatomy: ~5us head (window opens at the first
DMA issue; fine-grained first pieces put the first matmul at ~11.4us),
~109-112us PE stream (488 matmul slots; DR pairs ~2.16 slots each; ~8 cold
427ns matmuls while the HAM clock-gate releases; early feed jitter ~0-2.5us),
~10us tail (quarter drains+stores of the last o-tile, then the NRT-injected
per-semaphore clear epilogue S[3..255], ~6.4us, which partially overlaps
the final store waits since the kernel's own final barrier is dropped).

Hard-won constraints (measured, see memory notes): the chip occasionally
runs the whole stream at ~2.0GHz (+18% wall, run lottery, not code). HAM:
PE idle >3.4us mid-stream re-throttles to half clock; fixed-length PE
warm-ups can't straddle the variable stream start and backfire. The cores'
~220GB/s DMA share cannot carry a second fp8 copy of x (casts run on
DVE/ACT instead; gpsimd's CAST is 4x too slow). fp8 everywhere fails the
2e-2 gate (3.7e-2); one extra dense fp8 k-tile per row also fails.
DoublePixel is uint8-only. PSUM start flags zero whole 2KB banks. A dummy
activation without explicit bias= revives the (stripped) const-AP memsets
that would otherwise re-open the measured window ~1us early.
"""

import sys

if "/opt/trn_rl_repo" not in sys.path:
    sys.path.insert(0, "/opt/trn_rl_repo")

import numpy as np
import ml_dtypes

import concourse.bass as bass
import concourse.mybir as mybir
from concourse import tile
from concourse import bass_utils
from concourse.vector_clock import ScopedClock
from concourse.bass_utils import run_bass_kernel_spmd

# ---------------------------------------------------------------- constants
N_CORES = 8
NIN = 4096
NOUT = 4096
BW = 512
B, S = 4, 2048
TOK = B * S            # 8192 tokens
TPC = TOK // N_CORES   # 1024 tokens per core
P = 128                # partitions
NT = NOUT // P         # 32 output tiles of 128 rows
HALF = 512             # moving-operand free size per matmul (4-byte max)

XG = 4                 # k-tiles per x-group       (8KB/partition rows)
WG = 4                 # o-tiles per weight group
OG = 2                 # o-tiles per output store
NXG = NT // XG
NWG = NT // WG

# interior o-tiles compute their two triangular edge k-tiles (t-4, t+4) as
# one fp8 DoubleRow pair; the 7 dense k-tiles stay fp16
TI = list(range(4, 28))          # interior o-tiles
NTI = len(TI)                    # 24
# fp16 k-range per o-tile
KS16, KE16 = [], []
for t in range(NT):
    if 4 <= t <= 27:
        ks, ke = t - 3, t + 4
    else:
        ks, ke = max(0, t - BW // P), min(NT, t + BW // P + 1)
    KS16.append(ks)
    KE16.append(ke)
NK16 = [KE16[t] - KS16[t] for t in range(NT)]
WGNK = [sum(NK16[g * WG + i] for i in range(WG)) for g in range(NWG)]
WGOFF = [0] * NWG
for g in range(1, NWG):
    WGOFF[g] = WGOFF[g - 1] + WGNK[g - 1]
NK_TOTAL = sum(NK16)             # 220

COMPUTE_DT = mybir.dt.float16   # dense k-tiles: fp32-accumulated fp16
COMPUTE_NP = np.float16
FP8_DT = mybir.dt.float8e4      # triangle pairs: TRN FP8_EXP4 == e4m3 IEEE
FP8_NP = ml_dtypes.float8_e4m3
OUT_DT = mybir.dt.float16       # out <= ~184 abs, fp16 adds <5e-4 rel err
OUT_NP = np.float16
DR = mybir.MatmulPerfMode.DoubleRow

# ------------------------------------------------- walrus 1-wait workaround
_MAXW = 1


def _split_drain_and_barrier(self, tick_clock, wait_clock):
    nc = self.nc
    probe = nc.sync.nop(nofuse=True, hint="pre_drain_waits")
    wait_clock.add_sem_waits(probe.ins, ScopedClock({None: tick_clock.global_clock}))
    si = probe.ins.sync_info
    waits = list(si.on_wait) if si is not None and si.on_wait else []
    if len(waits) > _MAXW:
        probe.ins.sync_info = mybir.SyncInfo(
            on_wait=waits[:_MAXW],
            on_update=list(si.on_update) if si.on_update else [],
        )
        for i in range(_MAXW, len(waits), _MAXW):
            extra = nc.sync.nop(nofuse=True, hint=f"pre_drain_waits_{i}")
            extra.ins.sync_info = mybir.SyncInfo(
                on_wait=waits[i : i + _MAXW], on_update=[]
            )
    drain_inst = nc.sync.drain()
    wait_clock.add_sem_waits(
        drain_inst.ins, ScopedClock({None: tick_clock.global_clock})
    )
    dsi = drain_inst.ins.sync_info
    dwaits = list(dsi.on_wait) if dsi is not None and dsi.on_wait else []
    if len(dwaits) > _MAXW:
        # the NOPs above ran earlier on the same sequencer and carried them all
        drain_inst.ins.sync_info = mybir.SyncInfo(
            on_wait=[], on_update=list(dsi.on_update) if dsi.on_update else []
        )
    popped = nc._tile_sem_poison_stack.pop()
    assert popped is self._sem_poison
    self.sems.allocated()  # keep the pool bookkeeping consistent
    # no final all_engine_barrier and no kernel-side sem clears: the
    # NRT-injected epilogue already rendezvouses all engines on S[2] and
    # zeroes every semaphore S[3..255] before the NEFF completes, so our
    # own barrier + range-clear only added ~0.4us of serial tail


tile.TileContext._drain_and_barrier = _split_drain_and_barrier


def fix_multi_waits(nc: bass.Bass) -> None:
    """This walrus build allows only ONE sync wait per instruction. Carry
    extra waits on single-wait NOPs inserted just before, on the same
    engine/sequencer."""
    for bb in nc.m.functions[0].blocks:
        changed = False
        new_insts = []
        for inst in bb.instructions:
            si = inst.sync_info
            waits = list(si.on_wait) if si is not None and si.on_wait else []
            if len(waits) > 1:
                for w in waits[:-1]:
                    nop = mybir.InstNoOp(
                        name=nc.get_next_instruction_name(),
                        engine=inst.engine,
                        bass_nofuse=True,
                        sync_info=mybir.SyncInfo(on_wait=[w], on_update=[]),
                    )
                    new_insts.append(nop)
                inst.sync_info = mybir.SyncInfo(
                    on_wait=[waits[-1]],
                    on_update=list(si.on_update) if si.on_update else [],
                )
                changed = True
            new_insts.append(inst)
        if changed:
            bb.instructions = new_insts


def strip_dead_const_memsets(nc: bass.Bass) -> None:
    """Bass.__init__ memsets four const APs no instruction in this kernel
    reads. They are also the first instructions the profiler counts as
    "useful", so they start the measured window ~1us before the first DMA.
    Dead code — drop them from the IR."""
    for bb in nc.m.functions[0].blocks:
        bb.instructions = [
            inst
            for inst in bb.instructions
            if not (
                type(inst).__name__ == "InstMemSet"
                and inst.outs
                and str(getattr(inst.outs[0], "memref", "")).startswith("const-")
            )
        ]


# upload_artifacts reaches an internal blob store not present here; the trace
# path only needs the local files.
bass_utils.upload_artifacts = lambda tmpdir: "local://" + tmpdir


# ---------------------------------------------------------------- device IR
def build_program() -> bass.Bass:
    # Bass.__init__ ends with const-AP memsets + an all-engine barrier. The
    # consts are dead in this kernel and each engine's preamble is
    # program-ordered against its own body, while entry vs the previous
    # execution is gated by the NRT pseudo-barrier — skip the init barrier
    # (~3us off the preamble critical path); the memsets are stripped below.
    orig_barrier = bass.Bass.all_engine_barrier
    def _skip_init_barrier(self, *a, **kw):
        bass.Bass.all_engine_barrier = orig_barrier
        return None
    bass.Bass.all_engine_barrier = _skip_init_barrier
    try:
        nc = bass.Bass()
    finally:
        bass.Bass.all_engine_barrier = orig_barrier
    # all host-packed partition-major (see kernel())
    xpk = nc.declare_dram_parameter("xpk", [P, NT * TPC], COMPUTE_DT, isOutput=False)
    wpk = nc.declare_dram_parameter("wpk", [P, NK_TOTAL * P], COMPUTE_DT, isOutput=False)
    wpk8 = nc.declare_dram_parameter("wpk8", [P, NTI * 2 * P], FP8_DT, isOutput=False)
    bias = nc.declare_dram_parameter("bias_pk", [P, NT], mybir.dt.float32, isOutput=False)
    outp = nc.declare_dram_parameter("outpk", [P, NT * TPC], OUT_DT, isOutput=True)

    with tile.TileContext(nc) as tc:
        with (
            # fp16 x and W fit SBUF-resident; one buffer per group, no reuse
            tc.tile_pool(name="xp", bufs=1) as xp,
            tc.tile_pool(name="x8p", bufs=1) as x8p,
            tc.tile_pool(name="wp", bufs=1) as wp,
            tc.tile_pool(name="w8p", bufs=1) as w8p,
            tc.tile_pool(name="op", bufs=4) as op,
            tc.tile_pool(name="bp", bufs=1) as bp,
            # 4 bufs x (ps0+ps1) = all 8 PSUM banks: matmuls run up to 3
            # o-tiles ahead of the drains, absorbing drain jitter without a
            # PE stall (a >3.4us stall re-throttles the HAM to half clock)
            tc.tile_pool(name="pp", bufs=4, space="PSUM") as pp,
        ):
            x_tiles: list = [None] * NXG
            w_tiles: list = [None] * NWG
            x8t = None  # created below, captured by load_xg

            def load_xg(g, parts=1):
                # parts>1 fine-grains the load so early k-tile consumers can
                # start before the whole group lands (subtile deps)
                xt = xp.tile([P, XG * TPC], COMPUTE_DT, tag=f"xg{g}", name=f"xg{g}")
                base = g * XG * TPC
                step = XG * TPC // parts
                for i in range(parts):
                    nc.sync.dma_start(
                        out=xt[:, i * step : (i + 1) * step],
                        in_=xpk[:, base + i * step : base + (i + 1) * step],
                    )
                x_tiles[g] = xt
                # fp8 planes are derived on-device: the x8 HBM copy (4MB)
                # pushed the slowest core's total traffic past its ~220GB/s
                # share and starved the stream. gpsimd's CAST is too slow
                # (3.6us per k-tile, can't keep pace with one o-tile per
                # 3.4us), so the casts alternate between DVE (~0.7us) and
                # ACT (~0.9us), both of which have >60% idle time.
                for k in range(g * XG, (g + 1) * XG):
                    xs = xt[:, (k % XG) * TPC : ((k % XG) + 1) * TPC]
                    if k % 2 == 0:
                        nc.vector.tensor_copy(out=x8t[:, k, :], in_=xs)
                    else:
                        nc.scalar.copy(out=x8t[:, k, :], in_=xs)

            def load_wg(g, parts=1):
                wt = wp.tile(
                    [P, WGNK[g] * P], COMPUTE_DT, tag=f"wg{g}", name=f"wg{g}",
                )
                bounds = [WGNK[g] * i // parts for i in range(parts + 1)]
                base = WGOFF[g] * P
                for i in range(parts):
                    lo, hi = bounds[i] * P, bounds[i + 1] * P
                    # scalar-engine HWDGE queue: parallel to the sync queue,
                    # so w loads don't serialize behind x/out issue
                    nc.scalar.dma_start(
                        out=wt[:, lo:hi], in_=wpk[:, base + lo : base + hi]
                    )
                w_tiles[g] = wt

            x8t = x8p.tile([P, NT, TPC], FP8_DT, name="x8")

            # bias lands FIRST (16KB): the first DVE drain needs it at ~14us
            # and every o-tile's PSUM recycle depends on the drains — a core
            # whose bias queues behind megabytes of x can stall the PE for
            # >3.4us and trip a half-clock HAM re-throttle (observed: +6.5us
            # on the max core)
            bias_sb = bp.tile([P, NT], mybir.dt.float32)
            nc.scalar.dma_start(out=bias_sb[:, :], in_=bias[:, :])

            # the critical first bytes first, IN PARALLEL across the two
            # queues: o-tile 0 needs slab t0 (wg0 piece 1 of 5, on scalar)
            # and x k-tile 0 (xg0 piece 1 of 4, on sync) — one per queue so
            # they land concurrently (serializing both on the
            # earlier-starting scalar queue measured 2.5us WORSE: per-queue
            # bandwidth dominates the 1us preamble offset). wg1 is eager so
            # it rides scalar ahead of the fp8 weights; xg1 is split so k4
            # (o-tile 0's last unit) lands right behind k1-3.
            load_wg(0, parts=5)
            load_xg(0, parts=4)
            load_xg(1, parts=2)
            load_wg(1)

            # pre-warm the activation table: the lazily emitted
            # ACT_TABLE_LOAD costs 1.3us on the scalar queue and otherwise
            # lands right in front of the first PSUM drain. Explicit bias AP
            # (itself) so bass doesn't revive the dead const-AP memsets.
            warm = bp.tile([P, 1], mybir.dt.float32, name="actwarm")
            nc.scalar.activation(
                warm[:, :], warm[:, :],
                mybir.ActivationFunctionType.Identity, bias=warm[:, 0:1],
            )

            # fp8 weights: pairs 0-7 cover o-tiles 4-11; the rest load from
            # inside the loop, keeping the head queues lean
            w8t = w8p.tile([P, 2 * NTI, P], FP8_DT, name="w8")
            nc.scalar.dma_start(out=w8t[:, 0:16, :], in_=wpk8[:, 0 : 16 * P])

            # No PE warm-up matmuls: the HAM releases only after ~3.4us of
            # activity, and a fixed-length warmup either ends early (idle
            # >3.4us before the variable stream start -> mid-stream
            # re-throttle, measured +2us on one core) or runs long and
            # queues ahead of the real stream. The ~1.7us cold ramp on the
            # first 8 matmuls is the cheaper price.

            def k_max_needed(t):
                # interiors also read the fp8 cast of plane t+4
                return t + 4 if 4 <= t <= 27 else KE16[t] - 1

            ot = None
            for t in range(NT):
                gw = t // WG
                if w_tiles[gw] is None:
                    load_wg(gw)
                # prefetch x two o-tiles ahead: the t+4 plane's cast must
                # clear gpsimd before this o-tile's fp8 pair fires
                for g in range(KS16[t] // XG, k_max_needed(min(t + 2, NT - 1)) // XG + 1):
                    if x_tiles[g] is None:
                        load_xg(g)
                if t == 8:
                    nc.scalar.dma_start(
                        out=w8t[:, 16 : 2 * NTI, :], in_=wpk8[:, 16 * P : 2 * NTI * P]
                    )

                # slab offset of o-tile t inside its weight group
                off = sum(NK16[gw * WG + i] for i in range(t - gw * WG))
                wt = w_tiles[gw]
                nk = NK16[t]
                inner = 4 <= t <= 27
                bias_col = bias_sb[:, t : t + 1]

                if t != NT - 1:
                    ps0 = pp.tile([P, HALF], mybir.dt.float32, name=f"ps0_{t}", tag="ps0")
                    ps1 = pp.tile([P, HALF], mybir.dt.float32, name=f"ps1_{t}", tag="ps1")

                if t == NT - 1:
                    # quarter-split the final o-tile: each 256-col chain
                    # drains + stores while the next chain computes, so the
                    # kernel tail after the very last matmul is one 256-col
                    # drain + one 64KB store instead of a 512-col drain + a
                    # 256KB store. Quarters alternate PSUM banks (a start
                    # flag zeroes the whole 2KB bank, so a bank can only be
                    # restarted after its previous quarter drained — the
                    # drains run 2 quarters ahead, no stall).
                    ot = op.tile([P, TPC], OUT_DT, name=f"ot{t}", tag="ot")
                    Q = TPC // 4
                    for q in range(4):
                        qc = q * Q
                        # fresh pool buffer per quarter: recycling reaches
                        # back to o-tiles 29/30 whose drains finished long
                        # ago, instead of quarter q-2 whose drain would
                        # stall this chain's start
                        ps = pp.tile(
                            [P, HALF], mybir.dt.float32,
                            name=f"ps_q{q}", tag=("ps0" if q % 2 == 0 else "ps1"),
                        )
                        pq = ps[:, 0:Q]
                        for j in range(nk):
                            k = KS16[t] + j
                            lhsT = wt[:, (off + j) * P : (off + j + 1) * P]
                            xg = x_tiles[k // XG]
                            xb = (k % XG) * TPC
                            nc.tensor.matmul(
                                pq, lhsT, xg[:, xb + qc : xb + qc + Q],
                                start=(j == 0), stop=(j == nk - 1),
                            )
                        if q % 2 == 0:
                            nc.vector.tensor_scalar_add(
                                ot[:, qc : qc + Q], pq, bias_col
                            )
                            nc.sync.dma_start(
                                out=outp[:, t * TPC + qc : t * TPC + qc + Q],
                                in_=ot[:, qc : qc + Q],
                            )
                        else:
                            nc.scalar.activation(
                                ot[:, qc : qc + Q], pq,
                                mybir.ActivationFunctionType.Identity,
                                bias=bias_col,
                            )
                            nc.scalar.dma_start(
                                out=outp[:, t * TPC + qc : t * TPC + qc + Q],
                                in_=ot[:, qc : qc + Q],
                            )
                    continue

                for j in range(nk):
                    k = KS16[t] + j
                    lhsT = wt[:, (off + j) * P : (off + j + 1) * P]
                    xg = x_tiles[k // XG]
                    xb = (k % XG) * TPC
                    stop16 = (j == nk - 1) and not inner
                    nc.tensor.matmul(
                        ps0[:, :], lhsT, xg[:, xb : xb + HALF],
                        start=(j == 0), stop=stop16,
                    )
                    nc.tensor.matmul(
                        ps1[:, :], lhsT, xg[:, xb + HALF : xb + TPC],
                        start=(j == 0), stop=stop16,
                    )
                if inner:
                    # the two triangular edge k-tiles (t-4 upper, t+4 lower)
                    # as one fp8 DoubleRow pair: planes 8 k-tiles apart via a
                    # stride-8 slice of the x8 buffer
                    i = t - 4
                    l8 = w8t[:, 2 * i : 2 * i + 2, :]
                    nc.tensor.matmul(
                        ps0[:, :], l8, x8t[:, i : i + 9 : 8, 0:HALF],
                        start=False, stop=True, perf_mode=DR,
                    )
                    nc.tensor.matmul(
                        ps1[:, :], l8, x8t[:, i : i + 9 : 8, HALF:TPC],
                        start=False, stop=True, perf_mode=DR,
                    )

                # last o-tiles store individually, alternating queues, so the
                # tail's output DMA is spread instead of ending with one
                # batched store (shorter kernel tail)
                single = t >= NT - 4
                if single:
                    ot = op.tile([P, TPC], OUT_DT, name=f"ot{t}", tag="ot")
                    obase = 0
                elif t % OG == 0:
                    ot = op.tile([P, OG * TPC], OUT_DT, name=f"ot{t}", tag="ot")
                    obase = 0
                else:
                    obase = (t % OG) * TPC
                nc.vector.tensor_scalar_add(
                    ot[:, obase : obase + HALF], ps0[:, :], bias_col
                )
                nc.scalar.activation(
                    ot[:, obase + HALF : obase + TPC], ps1[:, :],
                    mybir.ActivationFunctionType.Identity, bias=bias_col,
                )
                if single:
                    eng = nc.scalar if t % 2 == 0 else nc.sync
                    eng.dma_start(
                        out=outp[:, t * TPC : (t + 1) * TPC],
                        in_=ot[:, 0:TPC],
                    )
                elif t % OG == OG - 1:
                    nc.sync.dma_start(
                        out=outp[:, (t - OG + 1) * TPC : (t + 1) * TPC],
                        in_=ot[:, :],
                    )

    strip_dead_const_memsets(nc)
    fix_multi_waits(nc)
    return nc


_PROGRAM_CACHE: bass.Bass | None = None


def _program() -> bass.Bass:
    global _PROGRAM_CACHE
    if _PROGRAM_CACHE is None:
        _PROGRAM_CACHE = build_program()
    return _PROGRAM_CACHE


# --------------------------------------------------------------- host side
def _pack_weights(W_values, rows, cols):
    W = np.zeros((NOUT, NIN), dtype=np.float32)
    W[rows, cols] = W_values
    slabs = []
    for t in range(NT):
        # slab[p, j*P + o] = W[t*P + o, (KS16[t]+j)*P + p]
        blk = W[t * P : (t + 1) * P, KS16[t] * P : KE16[t] * P]  # [o, nk*P]
        slab = blk.reshape(P, NK16[t], P).transpose(2, 1, 0).reshape(P, NK16[t] * P)
        slabs.append(slab)
    wpk = np.ascontiguousarray(np.concatenate(slabs, axis=1), dtype=COMPUTE_NP)

    w8 = np.zeros((P, NTI * 2 * P), dtype=np.float32)
    for i, t in enumerate(TI):
        for pl, k in enumerate((t - 4, t + 4)):
            # lhsT plane [p, o] = W[t*P + o, k*P + p]
            blk = W[t * P : (t + 1) * P, k * P : (k + 1) * P]  # [o, p]
            w8[:, (2 * i + pl) * P : (2 * i + pl + 1) * P] = blk.T
    wpk8 = w8.astype(FP8_NP)
    return wpk, wpk8


def kernel(x, W_values, bias, rows, cols, _trace=False):
    x = np.asarray(x, dtype=np.float32)
    W_values = np.asarray(W_values, dtype=np.float32)
    bias = np.asarray(bias, dtype=np.float32)
    rows = np.asarray(rows)
    cols = np.asarray(cols)

    x2d = x.reshape(TOK, NIN)
    wpk, wpk8 = _pack_weights(W_values, rows, cols)
    bias_pk = np.ascontiguousarray(bias.reshape(NT, P).T)

    in_maps = []
    for c in range(N_CORES):
        xs = x2d[c * TPC : (c + 1) * TPC, :]  # [TPC, NIN]
        # xpk[p, j*TPC + s] = xs[s, j*P + p]
        xpk = np.ascontiguousarray(
            xs.reshape(TPC, NT, P).transpose(2, 1, 0).reshape(P, NT * TPC),
            dtype=COMPUTE_NP,
        )
        in_maps.append({"xpk": xpk, "wpk": wpk, "wpk8": wpk8, "bias_pk": bias_pk})

    nc = _program()
    res = run_bass_kernel_spmd(
        nc, in_maps, core_ids=list(range(N_CORES)), trace=_trace,
        trace_cores=list(range(N_CORES)) if _trace else None,
    )

    out = np.empty((TOK, NOUT), dtype=np.float32)
    for c in range(N_CORES):
        outpk = res.results[c]["outpk"].astype(np.float32)  # [P, NT*TPC]
        # out[s, t*P + p] = outpk[p, t*TPC + s]
        out[c * TPC : (c + 1) * TPC, :] = (
            outpk.reshape(P, NT, TPC).transpose(2, 1, 0).reshape(TPC, NOUT)
        )
    out = out.reshape(B, S, NOUT)

    if _trace:
        kernel.last_exec_time_ns = res.exec_time_ns
        kernel.last_results = res
    return out


# revision 28
# speedup vs baseline: 1.0365x; 1.0365x over previous
"""Banded-matrix matmul kernel for Trainium2, SPMD over 8 NeuronCores.

Problem: out[b,s,o] = sum_i x[b,s,i] * W[o,i] + bias[o] with W a 4096x4096
band matrix (bandwidth 512 -> W[o,i] != 0 iff |o-i| <= 512), given in COO
form (W_values, rows, cols) with deterministic band ordering.

Strategy:
  - Host: densify W; shard tokens 8-way (data parallel; band + bias
    replicated). All device-side tensors are host-packed partition-major so
    every DMA is a 2D pattern with 6-8KB contiguous per-partition rows.
  - Device (per core): out.T[o,s] = W @ x.T per 128-row output tile,
    accumulating over the band's k-tiles (block tridiagonal in 512-blocks)
    in fp32 PSUM. Interior o-tiles (t=4..27) have exactly two TRIANGULAR
    k-tiles (t-4 upper-tri, t+4 lower-tri, ~129/1025 of each row's terms);
    those two are computed as ONE fp8e4 DoubleRow matmul pair (2 contraction
    planes packed along the free axis) while the 7 dense k-tiles stay fp16.
    Measured on the real data this puts rel err at 1.5e-2 (gate 2e-2) and
    removes 2 of 18 512-cycle PE passes per interior o-tile (~9% of the PE
    stream, more if DoubleRow really is 0.5 cycles/row as the cost model
    says). Output drains to fp16 (adds <5e-4 err), halving store traffic
    and the final-store landing that gates the kernel tail.
  - Host: unpack per-core [128, 32*1024] fp16 outputs back to [B, S, 4096]
    fp32.

Measured on 8 axon-tunneled trn2 cores: ~125-128us (vs 141us fp16
baseline), rel err 1.495e-2. Anatomy: ~5us head (window opens at the first
DMA issue; fine-grained first pieces put the first matmul at ~11.4us),
~109-112us PE stream (488 matmul slots; DR pairs ~2.16 slots each; ~8 cold
427ns matmuls while the HAM clock-gate releases; early feed jitter ~0-2.5us),
~10us tail (quarter drains+stores of the last o-tile, then the NRT-injected
per-semaphore clear epilogue S[3..255], ~6.4us, which partially overlaps
the final store waits since the kernel's own final barrier is dropped).

Hard-won constraints (measured, see memory notes): the chip occasionally
runs the whole stream at ~2.0GHz (+18% wall, run lottery, not code). HAM:
PE idle >3.4us mid-stream re-throttles to half clock; fixed-length PE
warm-ups can't straddle the variable stream start and backfire. The cores'
~220GB/s DMA share cannot carry a second fp8 copy of x (casts run on
DVE/ACT instead; gpsimd's CAST is 4x too slow). fp8 everywhere fails the
2e-2 gate (3.7e-2); one extra dense fp8 k-tile per row also fails.
DoublePixel is uint8-only. PSUM start flags zero whole 2KB banks. A dummy
activation without explicit bias= revives the (stripped) const-AP memsets
that would otherwise re-open the measured window ~1us early.
"""

import sys

if "/opt/trn_rl_repo" not in sys.path:
    sys.path.insert(0, "/opt/trn_rl_repo")

import numpy as np
import ml_dtypes

import concourse.bass as bass
import concourse.mybir as mybir
from concourse import tile
from concourse import bass_utils
from concourse.vector_clock import ScopedClock
from concourse.bass_utils import run_bass_kernel_spmd

# ---------------------------------------------------------------- constants
N_CORES = 8
NIN = 4096
NOUT = 4096
BW = 512
B, S = 4, 2048
TOK = B * S            # 8192 tokens
TPC = TOK // N_CORES   # 1024 tokens per core
P = 128                # partitions
NT = NOUT // P         # 32 output tiles of 128 rows
HALF = 512             # moving-operand free size per matmul (4-byte max)

XG = 4                 # k-tiles per x-group       (8KB/partition rows)
WG = 4                 # o-tiles per weight group
OG = 2                 # o-tiles per output store
NXG = NT // XG
NWG = NT // WG

# interior o-tiles compute their two triangular edge k-tiles (t-4, t+4) as
# one fp8 DoubleRow pair; the 7 dense k-tiles stay fp16
TI = list(range(4, 28))          # interior o-tiles
NTI = len(TI)                    # 24
# fp16 k-range per o-tile
KS16, KE16 = [], []
for t in range(NT):
    if 4 <= t <= 27:
        ks, ke = t - 3, t + 4
    else:
        ks, ke = max(0, t - BW // P), min(NT, t + BW // P + 1)
    KS16.append(ks)
    KE16.append(ke)
NK16 = [KE16[t] - KS16[t] for t in range(NT)]
WGNK = [sum(NK16[g * WG + i] for i in range(WG)) for g in range(NWG)]
WGOFF = [0] * NWG
for g in range(1, NWG):
    WGOFF[g] = WGOFF[g - 1] + WGNK[g - 1]
NK_TOTAL = sum(NK16)             # 220

COMPUTE_DT = mybir.dt.float16   # dense k-tiles: fp32-accumulated fp16
COMPUTE_NP = np.float16
FP8_DT = mybir.dt.float8e4      # triangle pairs: TRN FP8_EXP4 == e4m3 IEEE
FP8_NP = ml_dtypes.float8_e4m3
OUT_DT = mybir.dt.float16       # out <= ~184 abs, fp16 adds <5e-4 rel err
OUT_NP = np.float16
DR = mybir.MatmulPerfMode.DoubleRow

# ------------------------------------------------- walrus 1-wait workaround
_MAXW = 1


def _split_drain_and_barrier(self, tick_clock, wait_clock):
    nc = self.nc
    probe = nc.sync.nop(nofuse=True, hint="pre_drain_waits")
    wait_clock.add_sem_waits(probe.ins, ScopedClock({None: tick_clock.global_clock}))
    si = probe.ins.sync_info
    waits = list(si.on_wait) if si is not None and si.on_wait else []
    if len(waits) > _MAXW:
        probe.ins.sync_info = mybir.SyncInfo(
            on_wait=waits[:_MAXW],
            on_update=list(si.on_update) if si.on_update else [],
        )
        for i in range(_MAXW, len(waits), _MAXW):
            extra = nc.sync.nop(nofuse=True, hint=f"pre_drain_waits_{i}")
            extra.ins.sync_info = mybir.SyncInfo(
                on_wait=waits[i : i + _MAXW], on_update=[]
            )
    drain_inst = nc.sync.drain()
    wait_clock.add_sem_waits(
        drain_inst.ins, ScopedClock({None: tick_clock.global_clock})
    )
    dsi = drain_inst.ins.sync_info
    dwaits = list(dsi.on_wait) if dsi is not None and dsi.on_wait else []
    if len(dwaits) > _MAXW:
        # the NOPs above ran earlier on the same sequencer and carried them all
        drain_inst.ins.sync_info = mybir.SyncInfo(
            on_wait=[], on_update=list(dsi.on_update) if dsi.on_update else []
        )
    popped = nc._tile_sem_poison_stack.pop()
    assert popped is self._sem_poison
    self.sems.allocated()  # keep the pool bookkeeping consistent
    # no final all_engine_barrier and no kernel-side sem clears: the
    # NRT-injected epilogue already rendezvouses all engines on S[2] and
    # zeroes every semaphore S[3..255] before the NEFF completes, so our
    # own barrier + range-clear only added ~0.4us of serial tail


tile.TileContext._drain_and_barrier = _split_drain_and_barrier


def fix_multi_waits(nc: bass.Bass) -> None:
    """This walrus build allows only ONE sync wait per instruction. Carry
    extra waits on single-wait NOPs inserted just before, on the same
    engine/sequencer."""
    for bb in nc.m.functions[0].blocks:
        changed = False
        new_insts = []
        for inst in bb.instructions:
            si = inst.sync_info
            waits = list(si.on_wait) if si is not None and si.on_wait else []
            if len(waits) > 1:
                for w in waits[:-1]:
                    nop = mybir.InstNoOp(
                        name=nc.get_next_instruction_name(),
                        engine=inst.engine,
                        bass_nofuse=True,
                        sync_info=mybir.SyncInfo(on_wait=[w], on_update=[]),
                    )
                    new_insts.append(nop)
                inst.sync_info = mybir.SyncInfo(
                    on_wait=[waits[-1]],
                    on_update=list(si.on_update) if si.on_update else [],
                )
                changed = True
            new_insts.append(inst)
        if changed:
            bb.instructions = new_insts


def strip_dead_const_memsets(nc: bass.Bass) -> None:
    """Bass.__init__ memsets four const APs no instruction in this kernel
    reads. They are also the first instructions the profiler counts as
    "useful", so they start the measured window ~1us before the first DMA.
    Dead code — drop them from the IR."""
    for bb in nc.m.functions[0].blocks:
        bb.instructions = [
            inst
            for inst in bb.instructions
            if not (
                type(inst).__name__ == "InstMemSet"
                and inst.outs
                and str(getattr(inst.outs[0], "memref", "")).startswith("const-")
            )
        ]


# upload_artifacts reaches an internal blob store not present here; the trace
# path only needs the local files.
bass_utils.upload_artifacts = lambda tmpdir: "local://" + tmpdir


# ---------------------------------------------------------------- device IR
def build_program() -> bass.Bass:
    # Bass.__init__ ends with const-AP memsets + an all-engine barrier. The
    # consts are dead in this kernel and each engine's preamble is
    # program-ordered against its own body, while entry vs the previous
    # execution is gated by the NRT pseudo-barrier — skip the init barrier
    # (~3us off the preamble critical path); the memsets are stripped below.
    orig_barrier = bass.Bass.all_engine_barrier
    def _skip_init_barrier(self, *a, **kw):
        bass.Bass.all_engine_barrier = orig_barrier
        return None
    bass.Bass.all_engine_barrier = _skip_init_barrier
    try:
        nc = bass.Bass()
    finally:
        bass.Bass.all_engine_barrier = orig_barrier
    # all host-packed partition-major (see kernel())
    xpk = nc.declare_dram_parameter("xpk", [P, NT * TPC], COMPUTE_DT, isOutput=False)
    wpk = nc.declare_dram_parameter("wpk", [P, NK_TOTAL * P], COMPUTE_DT, isOutput=False)
    wpk8 = nc.declare_dram_parameter("wpk8", [P, NTI * 2 * P], FP8_DT, isOutput=False)
    bias = nc.declare_dram_parameter("bias_pk", [P, NT], mybir.dt.float32, isOutput=False)
    outp = nc.declare_dram_parameter("outpk", [P, NT * TPC], OUT_DT, isOutput=True)

    with tile.TileContext(nc) as tc:
        with (
            # fp16 x and W fit SBUF-resident; one buffer per group, no reuse
            tc.tile_pool(name="xp", bufs=1) as xp,
            tc.tile_pool(name="x8p", bufs=1) as x8p,
            tc.tile_pool(name="wp", bufs=1) as wp,
            tc.tile_pool(name="w8p", bufs=1) as w8p,
            tc.tile_pool(name="op", bufs=4) as op,
            tc.tile_pool(name="bp", bufs=1) as bp,
            # 4 bufs x (ps0+ps1) = all 8 PSUM banks: matmuls run up to 3
            # o-tiles ahead of the drains, absorbing drain jitter without a
            # PE stall (a >3.4us stall re-throttles the HAM to half clock)
            tc.tile_pool(name="pp", bufs=4, space="PSUM") as pp,
        ):
            x_tiles: list = [None] * NXG
            w_tiles: list = [None] * NWG
            x8t = None  # created below, captured by load_xg

            def load_xg(g, parts=1):
                # parts>1 fine-grains the load so early k-tile consumers can
                # start before the whole group lands (subtile deps)
                xt = xp.tile([P, XG * TPC], COMPUTE_DT, tag=f"xg{g}", name=f"xg{g}")
                base = g * XG * TPC
                step = XG * TPC // parts
                for i in range(parts):
                    nc.sync.dma_start(
                        out=xt[:, i * step : (i + 1) * step],
                        in_=xpk[:, base + i * step : base + (i + 1) * step],
                    )
                x_tiles[g] = xt

            # fp8 planes are derived on-device: the x8 HBM copy (4MB)
            # pushed the slowest core's total traffic past its ~220GB/s
            # share and starved the stream. gpsimd's CAST is too slow
            # (3.6us per k-tile), so casts alternate DVE (~0.7us) / ACT
            # (~0.9us). CRITICAL: the casts share those engines' FIFOs with
            # the PSUM drains, so a cast must only be emitted once its
            # source DMA has had ~2 o-tile periods to land — a cast stuck
            # waiting on a DMA blocks every later drain, stalls the PSUM
            # recycle, idles the PE >3.4us and trips a half-clock HAM
            # re-throttle on whichever core drew slow DMA that run.
            cast_done = [False] * NT

            def cast_plane(k):
                if cast_done[k]:
                    return
                cast_done[k] = True
                xt = x_tiles[k // XG]
                xs = xt[:, (k % XG) * TPC : ((k % XG) + 1) * TPC]
                if k % 2 == 0:
                    nc.vector.tensor_copy(out=x8t[:, k, :], in_=xs)
                else:
                    nc.scalar.copy(out=x8t[:, k, :], in_=xs)

            def load_wg(g, parts=1):
                wt = wp.tile(
                    [P, WGNK[g] * P], COMPUTE_DT, tag=f"wg{g}", name=f"wg{g}",
                )
                bounds = [WGNK[g] * i // parts for i in range(parts + 1)]
                base = WGOFF[g] * P
                for i in range(parts):
                    lo, hi = bounds[i] * P, bounds[i + 1] * P
                    # scalar-engine HWDGE queue: parallel to the sync queue,
                    # so w loads don't serialize behind x/out issue
                    nc.scalar.dma_start(
                        out=wt[:, lo:hi], in_=wpk[:, base + lo : base + hi]
                    )
                w_tiles[g] = wt

            x8t = x8p.tile([P, NT, TPC], FP8_DT, name="x8")

            # bias lands FIRST (16KB): the first DVE drain needs it at ~14us
            # and every o-tile's PSUM recycle depends on the drains — a core
            # whose bias queues behind megabytes of x can stall the PE for
            # >3.4us and trip a half-clock HAM re-throttle (observed: +6.5us
            # on the max core)
            bias_sb = bp.tile([P, NT], mybir.dt.float32)
            nc.scalar.dma_start(out=bias_sb[:, :], in_=bias[:, :])

            # the critical first bytes first, IN PARALLEL across the two
            # queues: o-tile 0 needs slab t0 (wg0 piece 1 of 5, on scalar)
            # and x k-tile 0 (xg0 piece 1 of 4, on sync) — one per queue so
            # they land concurrently (serializing both on the
            # earlier-starting scalar queue measured 2.5us WORSE: per-queue
            # bandwidth dominates the 1us preamble offset). wg1 is eager so
            # it rides scalar ahead of the fp8 weights; xg1 is split so k4
            # (o-tile 0's last unit) lands right behind k1-3.
            load_wg(0, parts=5)
            load_xg(0, parts=4)
            load_xg(1, parts=2)
            load_wg(1)

            # pre-warm the activation table: the lazily emitted
            # ACT_TABLE_LOAD costs 1.3us on the scalar queue and otherwise
            # lands right in front of the first PSUM drain. Explicit bias AP
            # (itself) so bass doesn't revive the dead const-AP memsets.
            warm = bp.tile([P, 1], mybir.dt.float32, name="actwarm")
            nc.scalar.activation(
                warm[:, :], warm[:, :],
                mybir.ActivationFunctionType.Identity, bias=warm[:, 0:1],
            )

            # fp8 weights: pairs 0-7 cover o-tiles 4-11; the rest load from
            # inside the loop, keeping the head queues lean
            w8t = w8p.tile([P, 2 * NTI, P], FP8_DT, name="w8")
            nc.scalar.dma_start(out=w8t[:, 0:16, :], in_=wpk8[:, 0 : 16 * P])

            # No PE warm-up matmuls: the HAM releases only after ~3.4us of
            # activity, and a fixed-length warmup either ends early (idle
            # >3.4us before the variable stream start -> mid-stream
            # re-throttle, measured +2us on one core) or runs long and
            # queues ahead of the real stream. The ~1.7us cold ramp on the
            # first 8 matmuls is the cheaper price.

            def k_max_needed(t):
                # interiors also read the fp8 cast of plane t+4
                return t + 4 if 4 <= t <= 27 else KE16[t] - 1

            ot = None
            for t in range(NT):
                gw = t // WG
                if w_tiles[gw] is None:
                    load_wg(gw)
                # prefetch x two o-tiles ahead of the fp16 need
                for g in range(KS16[t] // XG, k_max_needed(min(t + 2, NT - 1)) // XG + 1):
                    if x_tiles[g] is None:
                        load_xg(g)
                # cast schedule (see cast_plane): plane t+4 is consumed at
                # the END of this o-tile's chain and its group was loaded
                # >=2 tiles ago; planes 0-7 (only ever plane-A operands)
                # are cast two tiles before their first use at k+4
                if 2 <= t <= 9:
                    cast_plane(t - 2)
                if 4 <= t <= 27:
                    cast_plane(t + 4)
                if t == 8:
                    nc.scalar.dma_start(
                        out=w8t[:, 16 : 2 * NTI, :], in_=wpk8[:, 16 * P : 2 * NTI * P]
                    )

                # slab offset of o-tile t inside its weight group
                off = sum(NK16[gw * WG + i] for i in range(t - gw * WG))
                wt = w_tiles[gw]
                nk = NK16[t]
                inner = 4 <= t <= 27
                bias_col = bias_sb[:, t : t + 1]

                if t != NT - 1:
                    ps0 = pp.tile([P, HALF], mybir.dt.float32, name=f"ps0_{t}", tag="ps0")
                    ps1 = pp.tile([P, HALF], mybir.dt.float32, name=f"ps1_{t}", tag="ps1")

                if t == NT - 1:
                    # quarter-split the final o-tile: each 256-col chain
                    # drains + stores while the next chain computes, so the
                    # kernel tail after the very last matmul is one 256-col
                    # drain + one 64KB store instead of a 512-col drain + a
                    # 256KB store. Quarters alternate PSUM banks (a start
                    # flag zeroes the whole 2KB bank, so a bank can only be
                    # restarted after its previous quarter drained — the
                    # drains run 2 quarters ahead, no stall).
                    ot = op.tile([P, TPC], OUT_DT, name=f"ot{t}", tag="ot")
                    Q = TPC // 4
                    for q in range(4):
                        qc = q * Q
                        # fresh pool buffer per quarter: recycling reaches
                        # back to o-tiles 29/30 whose drains finished long
                        # ago, instead of quarter q-2 whose drain would
                        # stall this chain's start
                        ps = pp.tile(
                            [P, HALF], mybir.dt.float32,
                            name=f"ps_q{q}", tag=("ps0" if q % 2 == 0 else "ps1"),
                        )
                        pq = ps[:, 0:Q]
                        for j in range(nk):
                            k = KS16[t] + j
                            lhsT = wt[:, (off + j) * P : (off + j + 1) * P]
                            xg = x_tiles[k // XG]
                            xb = (k % XG) * TPC
                            nc.tensor.matmul(
                                pq, lhsT, xg[:, xb + qc : xb + qc + Q],
                                start=(j == 0), stop=(j == nk - 1),
                            )
                        if q % 2 == 0:
                            nc.vector.tensor_scalar_add(
                                ot[:, qc : qc + Q], pq, bias_col
                            )
                            nc.sync.dma_start(
                                out=outp[:, t * TPC + qc : t * TPC + qc + Q],
                                in_=ot[:, qc : qc + Q],
                            )
                        else:
                            nc.scalar.activation(
                                ot[:, qc : qc + Q], pq,
                                mybir.ActivationFunctionType.Identity,
                                bias=bias_col,
                            )
                            nc.scalar.dma_start(
                                out=outp[:, t * TPC + qc : t * TPC + qc + Q],
                                in_=ot[:, qc : qc + Q],
                            )
                    continue

                for j in range(nk):
                    k = KS16[t] + j
                    lhsT = wt[:, (off + j) * P : (off + j + 1) * P]
                    xg = x_tiles[k // XG]
                    xb = (k % XG) * TPC
                    stop16 = (j == nk - 1) and not inner
                    nc.tensor.matmul(
                        ps0[:, :], lhsT, xg[:, xb : xb + HALF],
                        start=(j == 0), stop=stop16,
                    )
                    nc.tensor.matmul(
                        ps1[:, :], lhsT, xg[:, xb + HALF : xb + TPC],
                        start=(j == 0), stop=stop16,
                    )
                if inner:
                    # the two triangular edge k-tiles (t-4 upper, t+4 lower)
                    # as one fp8 DoubleRow pair: planes 8 k-tiles apart via a
                    # stride-8 slice of the x8 buffer
                    i = t - 4
                    l8 = w8t[:, 2 * i : 2 * i + 2, :]
                    nc.tensor.matmul(
                        ps0[:, :], l8, x8t[:, i : i + 9 : 8, 0:HALF],
                        start=False, stop=True, perf_mode=DR,
                    )
                    nc.tensor.matmul(
                        ps1[:, :], l8, x8t[:, i : i + 9 : 8, HALF:TPC],
                        start=False, stop=True, perf_mode=DR,
                    )

                # last o-tiles store individually, alternating queues, so the
                # tail's output DMA is spread instead of ending with one
                # batched store (shorter kernel tail)
                single = t >= NT - 4
                if single:
                    ot = op.tile([P, TPC], OUT_DT, name=f"ot{t}", tag="ot")
                    obase = 0
                elif t % OG == 0:
                    ot = op.tile([P, OG * TPC], OUT_DT, name=f"ot{t}", tag="ot")
                    obase = 0
                else:
                    obase = (t % OG) * TPC
                nc.vector.tensor_scalar_add(
                    ot[:, obase : obase + HALF], ps0[:, :], bias_col
                )
                nc.scalar.activation(
                    ot[:, obase + HALF : obase + TPC], ps1[:, :],
                    mybir.ActivationFunctionType.Identity, bias=bias_col,
                )
                if single:
                    eng = nc.scalar if t % 2 == 0 else nc.sync
                    eng.dma_start(
                        out=outp[:, t * TPC : (t + 1) * TPC],
                        in_=ot[:, 0:TPC],
                    )
                elif t % OG == OG - 1:
                    nc.sync.dma_start(
                        out=outp[:, (t - OG + 1) * TPC : (t + 1) * TPC],
                        in_=ot[:, :],
                    )

    strip_dead_const_memsets(nc)
    fix_multi_waits(nc)
    return nc


_PROGRAM_CACHE: bass.Bass | None = None


def _program() -> bass.Bass:
    global _PROGRAM_CACHE
    if _PROGRAM_CACHE is None:
        _PROGRAM_CACHE = build_program()
    return _PROGRAM_CACHE


# --------------------------------------------------------------- host side
def _pack_weights(W_values, rows, cols):
    W = np.zeros((NOUT, NIN), dtype=np.float32)
    W[rows, cols] = W_values
    slabs = []
    for t in range(NT):
        # slab[p, j*P + o] = W[t*P + o, (KS16[t]+j)*P + p]
        blk = W[t * P : (t + 1) * P, KS16[t] * P : KE16[t] * P]  # [o, nk*P]
        slab = blk.reshape(P, NK16[t], P).transpose(2, 1, 0).reshape(P, NK16[t] * P)
        slabs.append(slab)
    wpk = np.ascontiguousarray(np.concatenate(slabs, axis=1), dtype=COMPUTE_NP)

    w8 = np.zeros((P, NTI * 2 * P), dtype=np.float32)
    for i, t in enumerate(TI):
        for pl, k in enumerate((t - 4, t + 4)):
            # lhsT plane [p, o] = W[t*P + o, k*P + p]
            blk = W[t * P : (t + 1) * P, k * P : (k + 1) * P]  # [o, p]
            w8[:, (2 * i + pl) * P : (2 * i + pl + 1) * P] = blk.T
    wpk8 = w8.astype(FP8_NP)
    return wpk, wpk8


def kernel(x, W_values, bias, rows, cols, _trace=False):
    x = np.asarray(x, dtype=np.float32)
    W_values = np.asarray(W_values, dtype=np.float32)
    bias = np.asarray(bias, dtype=np.float32)
    rows = np.asarray(rows)
    cols = np.asarray(cols)

    x2d = x.reshape(TOK, NIN)
    wpk, wpk8 = _pack_weights(W_values, rows, cols)
    bias_pk = np.ascontiguousarray(bias.reshape(NT, P).T)

    in_maps = []
    for c in range(N_CORES):
        xs = x2d[c * TPC : (c + 1) * TPC, :]  # [TPC, NIN]
        # xpk[p, j*TPC + s] = xs[s, j*P + p]
        xpk = np.ascontiguousarray(
            xs.reshape(TPC, NT, P).transpose(2, 1, 0).reshape(P, NT * TPC),
            dtype=COMPUTE_NP,
        )
        in_maps.append({"xpk": xpk, "wpk": wpk, "wpk8": wpk8, "bias_pk": bias_pk})

    nc = _program()
    res = run_bass_kernel_spmd(
        nc, in_maps, core_ids=list(range(N_CORES)), trace=_trace,
        trace_cores=list(range(N_CORES)) if _trace else None,
    )

    out = np.empty((TOK, NOUT), dtype=np.float32)
    for c in range(N_CORES):
        outpk = res.results[c]["outpk"].astype(np.float32)  # [P, NT*TPC]
        # out[s, t*P + p] = outpk[p, t*TPC + s]
        out[c * TPC : (c + 1) * TPC, :] = (
            outpk.reshape(P, NT, TPC).transpose(2, 1, 0).reshape(TPC, NOUT)
        )
    out = out.reshape(B, S, NOUT)

    if _trace:
        kernel.last_exec_time_ns = res.exec_time_ns
        kernel.last_results = res
    return out
